# revision 3
# baseline (speedup 1.0000x reference)
"""CBIndirectionLookup Trainium2 kernel, v2.

out[n] = results[argmax(all(x[n]==patterns))] = T[code(n)], code = sum_j x[n,j]*2^j.

Device pipeline per core (262144 elems), in macros of 8192 elems:
  1. DMA host-pretransposed xT (fp16 bit-planes; pure re-layout of x) + b7 plane.
  2. PE codes-matmul: lhsT = bitweights [128,16] fp16, rhs = xT [128,512]
     -> psum [16, 512] = c7 codes (bits 0..6; b7 handled by sign fold).
  3. ACT: psum -> SBUF fp16 code-rows [16, 512].
  4. Broadcast rows to all 128 partitions: SBUF->DRAM flatten, DRAM->SBUF seed
     [16, 8192], then 3 doubling copies -> bc[q, r*512+f] = code[r, f].
  5. DVE tensor_scalar is_equal (fp16 4x): one-hot [128 q, 8192].
  6. PE: 64x (LDWEIGHTS one-hot [128,128] + MATMUL luts [128,8] fp16)
     -> psum [128 e, (r,g2)*8] = P|Q per element.
  7. DVE recombine: out = Q + sigma*P (sigma = 1-2*b7), int32; DMA out.

Element id: n = ((((core*32 + m)*16 + r)*4 + g2)*128 + fc)
"""
import sys
sys.path.insert(0, "/opt/trn_rl_repo")

import numpy as np

N = 2_097_152
W_IN = 8
W_OUT = 4
N_CORES = 8
N_LOC = N // N_CORES            # 262144
MACRO = 8192                    # elements per macro tile
N_MACRO = N_LOC // MACRO        # 32
ROWS = 16                       # code rows per macro
CHUNK = 512                     # elements per code row
N_G2 = CHUNK // 128             # 4 ldweights groups per row


def _build_luts(patterns: np.ndarray, results: np.ndarray):
    """[P|Q] fp16 luts keyed by c7; P=(lo-hi)/2, Q=(lo+hi)/2 (b7 sign fold)."""
    pat2idx = {}
    for p in range(patterns.shape[0]):
        pat2idx[tuple(int(v) for v in patterns[p])] = p
    lo = np.zeros((128, W_OUT), np.float64)
    hi = np.zeros((128, W_OUT), np.float64)
    for q in range(128):
        bits = [(q >> j) & 1 for j in range(7)]
        lo[q] = results[pat2idx[tuple(bits + [0])]]
        hi[q] = results[pat2idx[tuple(bits + [1])]]
    Pm = (lo - hi) / 2.0
    Qm = (lo + hi) / 2.0
    rhs = np.concatenate([Pm, Qm], axis=1).astype(np.float16)  # [128, 8]
    assert np.array_equal(rhs.astype(np.float64), np.concatenate([Pm, Qm], 1))
    return rhs


def _build_bitweights():
    """lhsT [128, 16] fp16: W[r*8+j, r] = 2^j (j<7), 0 for j=7."""
    w = np.zeros((128, ROWS), np.float16)
    for r in range(ROWS):
        for j in range(7):
            w[r * 8 + j, r] = float(2 ** j)
    return w


def _build_kernel():
    import concourse.bass as bass
    import concourse.bacc as bacc
    import concourse.tile as tile
    from concourse import mybir

    nc = bacc.Bacc("TRN2", target_bir_lowering=False, debug=False,
                   num_devices=N_CORES)
    dt = mybir.dt

    xT = nc.dram_tensor("xT", [128, N_MACRO * CHUNK], dt.float16,
                        kind="ExternalInput").ap()
    b7 = nc.dram_tensor("b7", [128, N_MACRO * ROWS * N_G2], dt.float16,
                        kind="ExternalInput").ap()
    luts = nc.dram_tensor("luts", [128, 8], dt.float16,
                          kind="ExternalInput").ap()
    bw = nc.dram_tensor("bw", [128, ROWS], dt.float16,
                        kind="ExternalInput").ap()
    y = nc.dram_tensor("y", [128, N_LOC // 128 * W_OUT], dt.int32,
                       kind="ExternalOutput").ap()
    # per-macro code-row scratch (double buffered)
    scr = nc.dram_tensor("scr", [6, ROWS * 2 * CHUNK], dt.float16).ap()

    with tile.TileContext(nc) as tc:
        with (
            tc.tile_pool(name="const", bufs=1) as constp,
            tc.tile_pool(name="xin", bufs=3) as xinp,
            tc.tile_pool(name="crow", bufs=3) as crowp,
            tc.tile_pool(name="bc", bufs=5) as bcp,
            tc.tile_pool(name="sig", bufs=3) as sigp,
            tc.tile_pool(name="b7s", bufs=3) as b7p,
            tc.tile_pool(name="out", bufs=3) as outp,
            tc.tile_pool(name="pc", bufs=1, space="PSUM") as pcp,
            tc.tile_pool(name="pg", bufs=2, space="PSUM") as pgp,
        ):
            t_luts = constp.tile([128, 8], dt.float16)
            nc.sync.dma_start(out=t_luts[:], in_=luts[:])
            t_bw = constp.tile([128, ROWS], dt.float16)
            nc.sync.dma_start(out=t_bw[:], in_=bw[:])
            t_i32 = constp.tile([128, 1], dt.int32)
            nc.gpsimd.iota(t_i32[:], pattern=[[0, 1]], base=0,
                           channel_multiplier=1)
            t_iota = constp.tile([128, 1], dt.float32)
            nc.vector.tensor_copy(t_iota[:], t_i32[:])

            GROUP = 2                      # macros per broadcast group
            N_GROUP = N_MACRO // GROUP
            for g in range(N_GROUP):
                # --- inputs + codes for the group's 2 macros ---
                t_crow = crowp.tile([16, GROUP * CHUNK], dt.float16, tag="cr")
                t_xs, t_b7s, t_sgs = [], [], []
                for k in range(GROUP):
                    m = g * GROUP + k
                    t_x = xinp.tile([128, CHUNK], dt.float16, tag=f"x{k}")
                    nc.sync.dma_start(out=t_x[:],
                                      in_=xT[:, m * CHUNK:(m + 1) * CHUNK])
                    t_pc = pcp.tile([16, CHUNK], dt.float32, tag=f"pc{k}")
                    nc.tensor.matmul(t_pc[:], t_bw[:], t_x[:],
                                     start=True, stop=True)
                    nc.scalar.copy(t_crow[:, k * CHUNK:(k + 1) * CHUNK],
                                   t_pc[:])
                    t_b7 = b7p.tile([128, ROWS * N_G2], dt.float16,
                                    tag=f"b7{k}")
                    nc.sync.dma_start(
                        out=t_b7[:],
                        in_=b7[:, m * ROWS * N_G2:(m + 1) * ROWS * N_G2])
                    t_sg = sigp.tile([128, ROWS * N_G2], dt.float32,
                                     tag=f"sg{k}")
                    nc.vector.tensor_scalar(
                        out=t_sg[:], in0=t_b7[:], scalar1=-2.0, scalar2=1.0,
                        op0=mybir.AluOpType.mult, op1=mybir.AluOpType.add)
                    t_sgs.append(t_sg)

                # --- broadcast code rows [16, 1024] -> bc [128, 16384] ---
                # bc[q, r*1024 + k*512 + f] = code(row r, macro k, col f)
                GW = GROUP * ROWS * CHUNK
                nc.scalar.dma_start(
                    out=scr[g % 6].rearrange("(r f) -> r f", r=ROWS),
                    in_=t_crow[:])
                t_bc = bcp.tile([128, GW], dt.float16, tag="bc")
                H = GW // 2
                for h in range(2):
                    if g % 4 == 3 and h == 1:
                        # GPSIMD half-broadcast: off the DMA bus; latency
                        # hidden by the 5-deep bc pipeline
                        nc.gpsimd.dma_start(
                            out=t_bc[0:1, H:2 * H],
                            in_=scr[g % 6][None, H:2 * H])
                        nc.gpsimd.partition_broadcast(
                            t_bc[:, H:2 * H], t_bc[0:1, H:2 * H])
                    else:
                        q = nc.sync if (2 * g + h) % 2 == 0 else nc.scalar
                        q.dma_start(
                            out=t_bc[:, h * H:(h + 1) * H],
                            in_=scr[g % 6][None, h * H:(h + 1) * H]
                            .broadcast_to([128, H]))
                    nc.vector.tensor_scalar(
                        out=t_bc[:, h * H:(h + 1) * H],
                        in0=t_bc[:, h * H:(h + 1) * H], scalar1=t_iota[:],
                        scalar2=None, op0=mybir.AluOpType.is_equal)
                t_w = t_bc

                # --- gather matmuls + recombine per macro ---
                for k in range(GROUP):
                    m = g * GROUP + k
                    t_pg = pgp.tile([128, ROWS * N_G2 * 8], dt.float32,
                                    tag=f"pg{k}")
                    for r in range(ROWS):
                        for g2 in range(N_G2):
                            sl = (r * N_G2 + g2) * 8
                            col = r * GROUP * CHUNK + k * CHUNK + g2 * 128
                            nc.tensor.matmul(
                                t_pg[:, sl:sl + 8],
                                t_w[:, col:col + 128],
                                t_luts[:], start=True, stop=True)
                    pv = t_pg[:].rearrange("p (s w) -> p s w", w=8)
                    t_sg = t_sgs[k]
                    sg4 = bass.AP(tensor=t_sg.tensor, offset=t_sg[:].offset,
                                  ap=t_sg[:].ap + [[0, 4]])
                    t_t = outp.tile([128, ROWS * N_G2 * 4], dt.float32,
                                    tag=f"t{k}")
                    t3 = t_t[:].rearrange("p (s w) -> p s w", w=4)
                    nc.vector.tensor_tensor(out=t3[:, :, :], in0=pv[:, :, 0:4],
                                            in1=sg4, op=mybir.AluOpType.mult)
                    t_o = outp.tile([128, ROWS * N_G2 * 4], dt.int32,
                                    tag=f"o{k}")
                    o3 = t_o[:].rearrange("p (s w) -> p s w", w=4)
                    nc.vector.tensor_tensor(out=o3[:, :, :], in0=t3[:, :, :],
                                            in1=pv[:, :, 4:8],
                                            op=mybir.AluOpType.add)
                    nc.sync.dma_start(
                        out=y[:, m * ROWS * N_G2 * 4:(m + 1) * ROWS * N_G2 * 4],
                        in_=t_o[:])
    nc.compile()
    return nc


_CACHE = {}


def _host_layouts(x: np.ndarray):
    """xT fp16 bit-planes + b7 plane, per core. Pure re-layout of x."""
    xv = x.view(np.int16)[:, ::2]                       # [N, 8] int16 bits
    x6 = xv.reshape(N_CORES, N_MACRO, ROWS, N_G2, 128, W_IN)
    # xT[c][r*8+j, m*512 + g2*128 + fc]
    xT = np.ascontiguousarray(
        x6.transpose(0, 2, 5, 1, 3, 4)).astype(np.float16).reshape(
        N_CORES, ROWS * W_IN, N_MACRO * N_G2 * 128)
    # b7[c][fc, m*64 + r*4 + g2]
    b7 = np.ascontiguousarray(
        x6[..., 7].transpose(0, 4, 1, 2, 3)).astype(np.float16).reshape(
        N_CORES, 128, N_MACRO * ROWS * N_G2)
    return xT, b7


def kernel(x: np.ndarray, patterns: np.ndarray, results: np.ndarray) -> np.ndarray:
    import jax
    from jax.sharding import Mesh, PartitionSpec, NamedSharding
    from jax.experimental.shard_map import shard_map
    from concourse import mybir
    from concourse.bass2jax import (_bass_exec_p, install_neuronx_cc_hook,
                                    partition_id_tensor)

    x = np.asarray(x)
    patterns = np.asarray(patterns)
    results = np.asarray(results)
    rhs_luts = _build_luts(patterns, results)
    xT, b7 = _host_layouts(x)

    if "nc" not in _CACHE:
        _CACHE["nc"] = _build_kernel()
    nc = _CACHE["nc"]

    install_neuronx_cc_hook()
    partition_name = nc.partition_id_tensor.name if nc.partition_id_tensor else None
    in_names, out_names, out_avals, zero_outs = [], [], [], []
    for alloc in nc.m.functions[0].allocations:
        if not isinstance(alloc, mybir.MemoryLocationSet):
            continue
        name = alloc.memorylocations[0].name
        if alloc.kind == "ExternalInput":
            if name != partition_name:
                in_names.append(name)
        elif alloc.kind == "ExternalOutput":
            out_names.append(name)
            shape = tuple(alloc.tensor_shape)
            dtype = mybir.dt.np(alloc.dtype)
            out_avals.append(jax.core.ShapedArray(shape, dtype))
            zero_outs.append(np.zeros(shape, dtype))
    n_params = len(in_names)
    n_outs = len(out_avals)
    all_in_names = in_names + out_names + ([partition_name] if partition_name else [])

    def _body(*args):
        operands = list(args)
        if partition_name is not None:
            operands.append(partition_id_tensor())
        outs = _bass_exec_p.bind(
            *operands, out_avals=tuple(out_avals), in_names=tuple(all_in_names),
            out_names=tuple(out_names), lowering_input_output_aliases=(),
            sim_require_finite=False, sim_require_nnan=False, nc=nc)
        return tuple(outs)

    devices = jax.devices()[:N_CORES]
    mesh = Mesh(np.asarray(devices), ("core",))
    shard = NamedSharding(mesh, PartitionSpec("core"))
    fn = jax.jit(
        shard_map(_body, mesh=mesh,
                  in_specs=(PartitionSpec("core"),) * (n_params + n_outs),
                  out_specs=(PartitionSpec("core"),) * n_outs,
                  check_rep=False),
        keep_unused=True)

    arrays = {
        "xT": np.ascontiguousarray(xT.reshape(N_CORES * ROWS * W_IN, -1)),
        "b7": np.ascontiguousarray(b7.reshape(N_CORES * 128, -1)),
        "luts": np.broadcast_to(rhs_luts, (N_CORES, 128, 8)).reshape(
            N_CORES * 128, 8).copy(),
        "bw": np.broadcast_to(_build_bitweights(),
                              (N_CORES, 128, ROWS)).reshape(
            N_CORES * 128, ROWS).copy(),
    }
    args = [jax.device_put(arrays[nm], shard) for nm in in_names]
    args += [jax.device_put(
        np.zeros((N_CORES * z.shape[0], *z.shape[1:]), z.dtype), shard)
        for z in zero_outs]
    out_arrs = fn(*args)
    yi = out_names.index("y")
    yv = np.asarray(out_arrs[yi]).reshape(
        N_CORES, 128, N_MACRO, ROWS, N_G2, W_OUT)
    y_full = yv.transpose(0, 2, 3, 4, 1, 5).reshape(N, W_OUT)
    return np.ascontiguousarray(y_full).astype(np.int32)
